# revision 34
# baseline (speedup 1.0000x reference)
"""Fused attention layer (QKV projections + softmax(QK^T/sqrt(d))V) for
Trainium2, data-parallel over the batch across 8 NeuronCores.

Projection-free formulation (per core, one batch element, S=4096, D=512):
  scores^T = key (Wk^T Wq) query^T + v[k] 1^T + 1 u[q]^T (+ const); the
  per-query additive terms cancel in softmax, so only the per-key bias
  v = key (Wk^T bq) survives and rides through the ACT exp's per-partition
  bias.  With G = Wk^T Wq folded into the key side (KG^T = G^T key^T), the
  query projection disappears entirely.  On the value side,
  out = attn value Wv^T + bv (attn rows sum to one), so value is consumed
  in its natural layout with no transpose or projection; U^T = value^T exp^T
  accumulates on PSUM in two e-chunk passes (double-buffered 2-bank tiles),
  and Wv^T is applied per 128-query tile at the end, yielding the output in
  natural [q, e] layout.  The bias enters as rowsum[q]*bv via a K=1 matmul
  so the final 1/rowsum ACT scaling leaves exactly +bv.
  Value loads ride the scalar HWDGE queue and their fp16 casts, the lazy
  query-block loads/transposes and the previous block's epilogue are all
  software-pipelined into the main loop so the PE never waits.
All matmul operands fp16 (1 cyc/row, ~3e-4 rel err), accumulation fp32.
"""

import math

import numpy as np

S, D, P = 4096, 512, 128
NCORES = 8
KB = 512  # input/q block width


def build_attention(s=S, d=D, num_devices=NCORES):
    from contextlib import ExitStack

    import concourse.mybir as mybir
    import concourse.tile as tile
    from concourse import bacc
    from concourse.masks import make_identity

    f32 = mybir.dt.float32
    f16 = mybir.dt.float16
    Act = mybir.ActivationFunctionType

    dc = d // P        # d/e chunks (4)
    nkc = s // P       # key chunks (32)
    nqb = s // KB      # q blocks (8)
    tpb = KB // P      # 128-sub-blocks per block (4)
    lazy = nqb > 1 and nkc >= 8
    softmax_scale = 1.0 / math.sqrt(d)

    nc = bacc.Bacc(
        "TRN2", target_bir_lowering=False, debug=False, num_devices=num_devices
    )

    q_d = nc.dram_tensor("query", [s, d], f32, kind="ExternalInput").ap()
    k_d = nc.dram_tensor("key", [s, d], f32, kind="ExternalInput").ap()
    v_d = nc.dram_tensor("value", [s, d], f32, kind="ExternalInput").ap()
    wqn_d = nc.dram_tensor("wqn", [d, d], f16, kind="ExternalInput").ap()
    wkn_d = nc.dram_tensor("wkn", [d, d], f16, kind="ExternalInput").ap()
    wvt_d = nc.dram_tensor("wvt", [d, d], f16, kind="ExternalInput").ap()
    w1_d = nc.dram_tensor("w1c", [P, dc], f16, kind="ExternalInput").ap()
    bv_d = nc.dram_tensor("bv16", [1, d], f16, kind="ExternalInput").ap()
    out_d = nc.dram_tensor("out", [s, d], f32, kind="ExternalOutput").ap()

    with tile.TileContext(nc) as tc, ExitStack() as stack:
        consts = stack.enter_context(tc.tile_pool(name="consts", bufs=1))

        ident32 = consts.tile([P, P], f32, name="ident32")
        make_identity(nc, ident32)
        ident16 = consts.tile([P, P], f16, name="ident16")
        make_identity(nc, ident16)
        ones_col = consts.tile([P, 1], f32, name="ones_col")
        nc.vector.memset(ones_col, 1.0)

        wqn = consts.tile([P, dc, d], f16, name="wqn_sb")
        wkn = consts.tile([P, dc, d], f16, name="wkn_sb")
        wvt = consts.tile([P, dc, d], f16, name="wvt_sb")
        nc.scalar.dma_start(out=wqn, in_=wqn_d.rearrange("(c p) e -> p c e", p=P))
        nc.scalar.dma_start(out=wkn, in_=wkn_d.rearrange("(c p) e -> p c e", p=P))
        nc.scalar.dma_start(out=wvt, in_=wvt_d.rearrange("(c p) e -> p c e", p=P))
        w1c = consts.tile([P, dc], f16, name="w1c_sb")
        nc.scalar.dma_start(out=w1c, in_=w1_d)
        bv16 = consts.tile([1, d], f16, name="bv16_sb")
        nc.scalar.dma_start(out=bv16, in_=bv_d)

        # persistent activations
        qryt = consts.tile([P, dc, s], f16, name="qryt_sb")   # query^T [d, n]
        kgt = consts.tile([P, dc, s], f16, name="kgt_sb")     # (key G)^T [d', n]
        vnat = consts.tile([P, nkc, d], f16, name="vnat_sb")  # value [n, e]
        gsb = consts.tile([P, dc, d], f16, name="g_sb")       # G = Wk^T Wq
        vb = consts.tile([P, nkc], f32, name="vb_sb")         # scale * key@w1

        stage = stack.enter_context(tc.tile_pool(name="stage", bufs=4))
        ps_st = stack.enter_context(tc.tile_pool(name="ps_st", bufs=2, space="PSUM"))
        ps_small = stack.enter_context(
            tc.tile_pool(name="ps_small", bufs=2, space="PSUM")
        )

        HB = tpb // 2  # rows per half-load (2 sub-blocks of 128)

        def load_block(x_d, nb):
            halves = []
            for half in range(2):
                xh = stage.tile([P, HB, d], f32, tag="x_nat")
                nc.sync.dma_start(
                    out=xh,
                    in_=x_d[
                        nb * KB + half * HB * P : nb * KB + (half + 1) * HB * P, :
                    ].rearrange("(s p) d -> p s d", p=P),
                )
                halves.append(xh)
            return halves

        def emit_cast16(halves, pool16):
            # fp32 -> fp16 cast of a staged block, one half on ACT one on DVE
            h0 = pool16.tile([P, HB, d], f16, tag="x16")
            h1 = pool16.tile([P, HB, d], f16, tag="x16")
            nc.scalar.copy(out=h0, in_=halves[0])
            nc.vector.tensor_copy(out=h1, in_=halves[1])
            return [h0, h1]

        def emit_transpose_si(h16, dst, col0, si):
            xh = h16[si // HB]
            pt = ps_small.tile([P, dc, P], f16, tag="ps_small")
            for c in range(dc):
                nc.tensor.transpose(
                    pt[:, c, :], xh[:, si % HB, c * P : (c + 1) * P], ident16
                )
            nc.vector.tensor_copy(
                out=dst[:, :, col0 + si * P : col0 + (si + 1) * P], in_=pt
            )

        def emit_transposes(h16, dst, nb):
            for si in range(tpb):
                emit_transpose_si(h16, dst, nb * KB, si)

        def emit_vcast(halves, nb):
            # fp16 cast of a value block into vnat, split ACT/DVE
            nc.scalar.copy(out=vnat[:, nb * tpb : nb * tpb + HB, :], in_=halves[0])
            nc.vector.tensor_copy(
                out=vnat[:, nb * tpb + HB : (nb + 1) * tpb, :], in_=halves[1]
            )

        # value half-loads ride the scalar queue (emitted just-in-time so they
        # never starve the key/query loads on the sync queue)
        vhalves = [None] * nqb

        def load_vblock(nb):
            halves = []
            for half in range(2):
                xh = stage.tile([P, HB, d], f32, tag="xv_nat")
                nc.scalar.dma_start(
                    out=xh,
                    in_=v_d[
                        nb * KB + half * HB * P : nb * KB + (half + 1) * HB * P, :
                    ].rearrange("(s p) d -> p s d", p=P),
                )
                halves.append(xh)
            vhalves[nb] = halves

        # ---------------- Phase 1: key side (keyt is transient) ----------------
        with (
            tc.tile_pool(name="keyt_pool", bufs=1) as keyt_pool,
            tc.tile_pool(name="stage16k", bufs=4) as stage16k,
        ):
            keyt = keyt_pool.tile([P, dc, s], f16, name="keyt_sb")

            def emit_kg(nb):
                # KG^T block: lhsT = G chunks, rhs = keyT block
                for ec in range(dc):
                    pp = ps_st.tile([P, KB], f32, tag="psum_st")
                    for c in range(dc):
                        nc.tensor.matmul(
                            pp,
                            gsb[:, c, ec * P : (ec + 1) * P],
                            keyt[:, c, nb * KB : (nb + 1) * KB],
                            start=(c == 0),
                            stop=(c == dc - 1),
                        )
                    nc.scalar.copy(out=kgt[:, ec, nb * KB : (nb + 1) * KB], in_=pp)
                # v-vector chunks: v[k] = scale * key @ (Wk^T bq)
                for si in range(tpb):
                    vp = ps_small.tile([P, 1], f32, tag="ps_small")
                    for c in range(dc):
                        nc.tensor.matmul(
                            vp,
                            keyt[:, c, nb * KB + si * P : nb * KB + (si + 1) * P],
                            w1c[:, c : c + 1],
                            start=(c == 0),
                            stop=(c == dc - 1),
                        )
                    nc.vector.tensor_copy(
                        out=vb[:, nb * tpb + si : nb * tpb + si + 1], in_=vp
                    )

            for nb in range(nqb):
                halves = load_block(k_d, nb)
                if nb == 0:
                    # transpose straight from fp32 staging (2 cyc/row) so the
                    # first PE op does not wait on the ACT cast chain
                    for si in range(tpb):
                        xh = halves[si // HB]
                        pt = ps_small.tile([P, dc, P], f32, tag="ps_small")
                        for c in range(dc):
                            nc.tensor.transpose(
                                pt[:, c, :],
                                xh[:, si % HB, c * P : (c + 1) * P],
                                ident32,
                            )
                        nc.vector.tensor_copy(
                            out=keyt[:, :, si * P : (si + 1) * P], in_=pt
                        )
                else:
                    h16 = emit_cast16(halves, stage16k)
                    emit_transposes(h16, keyt, nb)
                if nb == 0:
                    # G = Wk^T Wq (emitted after the first transposes so the
                    # PE can start before the weight DMAs land)
                    for dch in range(dc):
                        gp = ps_st.tile([P, d], f32, tag="psum_st")
                        for e in range(dc):
                            nc.tensor.matmul(
                                gp,
                                wkn[:, e, dch * P : (dch + 1) * P],
                                wqn[:, e, :],
                                start=(e == 0),
                                stop=(e == dc - 1),
                            )
                        nc.scalar.copy(out=gsb[:, dch, :], in_=gp)
                emit_kg(nb)

        # ---------------- Phase 2: attention (scores transposed) ----------------
        with (
            tc.tile_pool(name="expt_pool", bufs=nkc) as expt_pool,
            tc.tile_pool(name="rsum_pool", bufs=2) as rsum_pool,
            tc.tile_pool(name="unsb_pool", bufs=2) as unsb_pool,
            tc.tile_pool(name="osb_pool", bufs=3) as osb_pool,
            tc.tile_pool(name="stat_pool", bufs=2) as stat_pool,
            tc.tile_pool(name="stage16q", bufs=2) as stage16q,
            tc.tile_pool(name="ps_ut", bufs=2, space="PSUM") as ps_ut,
        ):
            # query block 0 + value block 0 before the main loop
            qhalves = load_block(q_d, 0)
            emit_transposes(emit_cast16(qhalves, stage16q), qryt, 0)
            for nb in range(min(3, nqb)):
                load_vblock(nb)
            emit_vcast(vhalves[0], 0)
            if not lazy:
                for nb in range(1, nqb):
                    emit_transposes(
                        emit_cast16(load_block(q_d, nb), stage16q), qryt, nb
                    )
                for nb in range(3, nqb):
                    load_vblock(nb)
                for nb in range(1, nqb):
                    emit_vcast(vhalves[nb], nb)

            def emit_output(qb, un_sb, rs16, recip_row):
                for qs in range(tpb):
                    rc_ps = ps_small.tile([P, 1], f32, tag="ps_small")
                    nc.tensor.transpose(
                        rc_ps,
                        recip_row[0:1, qs * P : (qs + 1) * P],
                        ident32[0:1, 0:1],
                    )
                    rc = stat_pool.tile([P, 1], f32, tag="rc")
                    nc.vector.tensor_copy(out=rc, in_=rc_ps)
                    po = ps_small.tile([P, d], f32, tag="ps_small")
                    for c in range(dc):
                        nc.tensor.matmul(
                            po,
                            un_sb[:, c, qs * P : (qs + 1) * P],
                            wvt[:, c, :],
                            start=(c == 0),
                            stop=False,
                        )
                    nc.tensor.matmul(
                        po,
                        rs16[0:1, qs * P : (qs + 1) * P],
                        bv16,
                        start=False,
                        stop=True,
                    )
                    out_sb = osb_pool.tile([P, d], f32, tag="out_sb")
                    nc.scalar.activation(
                        out=out_sb, in_=po, func=Act.Identity, scale=rc[:, 0:1]
                    )
                    nc.sync.dma_start(
                        out=out_d[qb * KB + qs * P : qb * KB + (qs + 1) * P, :],
                        in_=out_sb,
                    )

            pending = None
            next_q = None
            for qb in range(nqb):
                rsum = rsum_pool.tile([P, KB], f32, tag="rsum")
                ut_a = ps_ut.tile([P, 2, KB], f32, tag="ut")
                un_sb = unsb_pool.tile([P, dc, KB], f16, tag="un_sb")
                expts = []
                for kc in range(nkc):
                    if lazy and qb == 0 and kc >= 3 and kc % 3 == 0:
                        b = kc // 3
                        if b <= nqb - 1:
                            if b + 2 <= nqb - 1:
                                load_vblock(b + 2)
                            emit_vcast(vhalves[b], b)
                    psum_st = ps_st.tile([P, KB], f32, tag="psum_st")
                    for ec in range(dc):
                        nc.tensor.matmul(
                            psum_st,
                            kgt[:, ec, kc * P : (kc + 1) * P],
                            qryt[:, ec, qb * KB : (qb + 1) * KB],
                            start=(ec == 0),
                            stop=(ec == dc - 1),
                        )
                    expt = expt_pool.tile([P, KB], f16, tag="expt")
                    expts.append(expt)
                    nc.scalar.activation(
                        out=expt,
                        in_=psum_st,
                        func=Act.Exp,
                        scale=softmax_scale,
                        bias=vb[:, kc : kc + 1],
                    )
                    if kc == 0:
                        nc.vector.tensor_copy(out=rsum, in_=expt)
                    else:
                        nc.vector.tensor_add(rsum, rsum, expt)
                    for ec in range(2):
                        nc.tensor.matmul(
                            ut_a[:, ec, :],
                            vnat[:, kc, ec * P : (ec + 1) * P],
                            expt,
                            start=(kc == 0),
                            stop=(kc == nkc - 1),
                        )
                    if lazy:
                        if kc == 0 and qb + 1 < nqb:
                            next_q = load_block(q_d, qb + 1)
                        if kc == 1 and pending is not None:
                            emit_output(*pending)
                            pending = None
                        if kc == 3 and qb + 1 < nqb:
                            next_q16 = emit_cast16(next_q, stage16q)
                        if 4 <= kc <= 7 and qb + 1 < nqb:
                            emit_transpose_si(
                                next_q16, qryt, (qb + 1) * KB, kc - 4
                            )
                    elif kc == 1 and pending is not None:
                        emit_output(*pending)
                        pending = None
                # drain pass-A psum early (frees its slot for the next block)
                nc.vector.tensor_copy(out=un_sb[:, 0:2, :], in_=ut_a)
                # row-sums + reciprocal (overlap with pass B)
                rs_ps = ps_small.tile([1, KB], f32, tag="ps_small")
                nc.tensor.matmul(rs_ps, ones_col, rsum, start=True, stop=True)
                recip_row = stat_pool.tile([1, KB], f32, tag="recip_row")
                nc.vector.reciprocal(out=recip_row, in_=rs_ps)
                rs16 = stat_pool.tile([1, KB], f16, tag="rs16")
                nc.vector.tensor_copy(out=rs16, in_=rs_ps)
                # pass B: e-chunks 2,3 over the stored exp tiles
                ut_b = ps_ut.tile([P, 2, KB], f32, tag="ut")
                for kc in range(nkc):
                    for ec in range(2):
                        nc.tensor.matmul(
                            ut_b[:, ec, :],
                            vnat[:, kc, (2 + ec) * P : (3 + ec) * P],
                            expts[kc],
                            start=(kc == 0),
                            stop=(kc == nkc - 1),
                        )
                # drain pass-B psum, split DVE/ACT
                nc.vector.tensor_copy(out=un_sb[:, 2:3, :], in_=ut_b[:, 0:1, :])
                nc.scalar.copy(out=un_sb[:, 3:4, :], in_=ut_b[:, 1:2, :])
                pending = (qb, un_sb, rs16, recip_row)
            emit_output(*pending)

    nc.compile()
    return nc


_CACHE = {}


def _get_nc():
    if "nc" not in _CACHE:
        _CACHE["nc"] = build_attention()
    return _CACHE["nc"]


def _in_maps(query, key, value, Wq, bq, Wk, bk, Wv, bv, n_cores=NCORES):
    Wq = np.asarray(Wq, np.float32)
    Wk = np.asarray(Wk, np.float32)
    Wv = np.asarray(Wv, np.float32)
    bq = np.asarray(bq, np.float32)
    bv = np.asarray(bv, np.float32)
    wqn = Wq.astype(np.float16)
    wkn = Wk.astype(np.float16)
    wvt = np.ascontiguousarray(Wv.T).astype(np.float16)
    scale = 1.0 / math.sqrt(D)
    w1 = (scale * (Wk.T @ bq)).astype(np.float16)  # [D]
    dcn = D // P
    w1c = np.ascontiguousarray(w1.reshape(dcn, P).T)  # [P, dc]
    bv16 = bv.astype(np.float16).reshape(1, D)
    query = np.asarray(query, np.float32)
    key = np.asarray(key, np.float32)
    value = np.asarray(value, np.float32)
    return [
        {
            "query": query[i],
            "key": key[i],
            "value": value[i],
            "wqn": wqn,
            "wkn": wkn,
            "wvt": wvt,
            "w1c": w1c,
            "bv16": bv16,
        }
        for i in range(n_cores)
    ]


def _build_runner():
    """Compile once and return a callable(in_maps) -> [out per core].

    Same lowering as concourse.bass2jax.run_bass_via_pjrt, but the
    jitted shard_map executable is cached so repeat kernel() calls skip
    retracing/recompiling.
    """
    import jax
    import concourse.mybir as mybir
    from concourse import bass2jax
    from jax.experimental.shard_map import shard_map
    from jax.sharding import Mesh, PartitionSpec

    bass2jax.install_neuronx_cc_hook()
    nc = _get_nc()
    partition_name = nc.partition_id_tensor.name if nc.partition_id_tensor else None
    in_names, out_names, out_avals, zero_templates = [], [], [], []
    for alloc in nc.m.functions[0].allocations:
        if not isinstance(alloc, mybir.MemoryLocationSet):
            continue
        name = alloc.memorylocations[0].name
        if alloc.kind == "ExternalInput":
            if name != partition_name:
                in_names.append(name)
        elif alloc.kind == "ExternalOutput":
            shape = tuple(alloc.tensor_shape)
            dtype = mybir.dt.np(alloc.dtype)
            out_names.append(name)
            out_avals.append(jax.core.ShapedArray(shape, dtype))
            zero_templates.append((shape, dtype))
    n_params = len(in_names)
    n_outs = len(out_names)
    all_in_names = list(in_names) + list(out_names)
    if partition_name is not None:
        all_in_names.append(partition_name)
    donate = tuple(range(n_params, n_params + n_outs))

    def _body(*args):
        operands = list(args)
        if partition_name is not None:
            operands.append(bass2jax.partition_id_tensor())
        outs = bass2jax._bass_exec_p.bind(
            *operands,
            out_avals=tuple(out_avals),
            in_names=tuple(all_in_names),
            out_names=tuple(out_names),
            lowering_input_output_aliases=(),
            sim_require_finite=True,
            sim_require_nnan=True,
            nc=nc,
        )
        return tuple(outs)

    devices = jax.devices()[:NCORES]
    mesh = Mesh(np.asarray(devices), ("core",))
    in_specs = (PartitionSpec("core"),) * (n_params + n_outs)
    out_specs = (PartitionSpec("core"),) * n_outs
    sharded = jax.jit(
        shard_map(
            _body, mesh=mesh, in_specs=in_specs, out_specs=out_specs, check_rep=False
        ),
        donate_argnums=donate,
        keep_unused=True,
    )

    def run(in_maps):
        concat_in = [
            np.concatenate([np.asarray(m[name]) for m in in_maps], axis=0)
            for name in in_names
        ]
        concat_zeros = [
            np.zeros((NCORES * shp[0], *shp[1:]), dt) for shp, dt in zero_templates
        ]
        out_arrs = sharded(*concat_in, *concat_zeros)
        out = np.asarray(out_arrs[out_names.index("out")])
        return out.reshape(NCORES, S, D)

    return run


def _get_runner():
    if "run" not in _CACHE:
        _CACHE["run"] = _build_runner()
    return _CACHE["run"]


def kernel(query, key, value, Wq, bq, Wk, bk, Wv, bv):
    run = _get_runner()
    in_maps = _in_maps(query, key, value, Wq, bq, Wk, bk, Wv, bv)
    return run(in_maps)


# revision 38
# speedup vs baseline: 1.0111x; 1.0111x over previous
"""Fused attention layer (QKV projections + softmax(QK^T/sqrt(d))V) for
Trainium2, data-parallel over the batch across 8 NeuronCores.

Projection-free formulation (per core, one batch element, S=4096, D=512):
  scores^T = key (Wk^T Wq) query^T + v[k] 1^T + 1 u[q]^T (+ const); the
  per-query additive terms cancel in softmax, so only the per-key bias
  v = key (Wk^T bq) survives and rides through the ACT exp's per-partition
  bias.  With G = Wk^T Wq folded into the key side (KG^T = G^T key^T), the
  query projection disappears entirely.  On the value side,
  out = attn value Wv^T + bv (attn rows sum to one), so value is consumed
  in its natural layout with no transpose or projection; U^T = value^T exp^T
  accumulates on PSUM in two e-chunk passes (double-buffered 2-bank tiles),
  and Wv^T is applied per 128-query tile at the end, yielding the output in
  natural [q, e] layout.  The bias enters as rowsum[q]*bv via a K=1 matmul
  so the final 1/rowsum ACT scaling leaves exactly +bv.
  Value loads ride the scalar HWDGE queue and their fp16 casts, the lazy
  query-block loads/transposes and the previous block's epilogue are all
  software-pipelined into the main loop so the PE never waits.
All matmul operands fp16 (1 cyc/row, ~3e-4 rel err), accumulation fp32.
"""

import math

import numpy as np

S, D, P = 4096, 512, 128
NCORES = 8
KB = 512  # input/q block width


def build_attention(s=S, d=D, num_devices=NCORES):
    from contextlib import ExitStack

    import concourse.mybir as mybir
    import concourse.tile as tile
    from concourse import bacc
    from concourse.masks import make_identity

    f32 = mybir.dt.float32
    f16 = mybir.dt.float16
    Act = mybir.ActivationFunctionType

    dc = d // P        # d/e chunks (4)
    nkc = s // P       # key chunks (32)
    nqb = s // KB      # q blocks (8)
    tpb = KB // P      # 128-sub-blocks per block (4)
    lazy = nqb > 1 and nkc >= 8
    softmax_scale = 1.0 / math.sqrt(d)

    nc = bacc.Bacc(
        "TRN2", target_bir_lowering=False, debug=False, num_devices=num_devices
    )

    q_d = nc.dram_tensor("query", [s, d], f32, kind="ExternalInput").ap()
    k_d = nc.dram_tensor("key", [s, d], f32, kind="ExternalInput").ap()
    v_d = nc.dram_tensor("value", [s, d], f32, kind="ExternalInput").ap()
    wqn_d = nc.dram_tensor("wqn", [d, d], f16, kind="ExternalInput").ap()
    wkn_d = nc.dram_tensor("wkn", [d, d], f16, kind="ExternalInput").ap()
    wvt_d = nc.dram_tensor("wvt", [d, d], f16, kind="ExternalInput").ap()
    w1_d = nc.dram_tensor("w1c", [P, dc], f16, kind="ExternalInput").ap()
    bv_d = nc.dram_tensor("bv16", [1, d], f16, kind="ExternalInput").ap()
    out_d = nc.dram_tensor("out", [s, d], f32, kind="ExternalOutput").ap()

    with tile.TileContext(nc) as tc, ExitStack() as stack:
        consts = stack.enter_context(tc.tile_pool(name="consts", bufs=1))

        ident32 = consts.tile([P, P], f32, name="ident32")
        make_identity(nc, ident32)
        ident16 = consts.tile([P, P], f16, name="ident16")
        make_identity(nc, ident16)
        ones_col = consts.tile([P, 1], f16, name="ones_col")
        nc.vector.memset(ones_col, 1.0)

        wqn = consts.tile([P, dc, d], f16, name="wqn_sb")
        wkn = consts.tile([P, dc, d], f16, name="wkn_sb")
        wvt = consts.tile([P, dc, d], f16, name="wvt_sb")
        nc.scalar.dma_start(out=wqn, in_=wqn_d.rearrange("(c p) e -> p c e", p=P))
        nc.scalar.dma_start(out=wkn, in_=wkn_d.rearrange("(c p) e -> p c e", p=P))
        nc.scalar.dma_start(out=wvt, in_=wvt_d.rearrange("(c p) e -> p c e", p=P))
        w1c = consts.tile([P, dc], f16, name="w1c_sb")
        nc.scalar.dma_start(out=w1c, in_=w1_d)
        bv16 = consts.tile([1, d], f16, name="bv16_sb")
        nc.scalar.dma_start(out=bv16, in_=bv_d)

        # persistent activations
        qryt = consts.tile([P, dc, s], f16, name="qryt_sb")   # query^T [d, n]
        kgt = consts.tile([P, dc, s], f16, name="kgt_sb")     # (key G)^T [d', n]
        vnat = consts.tile([P, nkc, d], f16, name="vnat_sb")  # value [n, e]
        gsb = consts.tile([P, dc, d], f16, name="g_sb")       # G = Wk^T Wq
        vb = consts.tile([P, nkc], f32, name="vb_sb")         # scale * key@w1

        stage = stack.enter_context(tc.tile_pool(name="stage", bufs=4))
        ps_st = stack.enter_context(tc.tile_pool(name="ps_st", bufs=2, space="PSUM"))
        ps_small = stack.enter_context(
            tc.tile_pool(name="ps_small", bufs=2, space="PSUM")
        )

        HB = tpb // 2  # rows per half-load (2 sub-blocks of 128)

        def load_block(x_d, nb):
            halves = []
            for half in range(2):
                xh = stage.tile([P, HB, d], f32, tag="x_nat")
                nc.sync.dma_start(
                    out=xh,
                    in_=x_d[
                        nb * KB + half * HB * P : nb * KB + (half + 1) * HB * P, :
                    ].rearrange("(s p) d -> p s d", p=P),
                )
                halves.append(xh)
            return halves

        def emit_cast16(halves, pool16):
            # fp32 -> fp16 cast of a staged block, one half on ACT one on DVE
            h0 = pool16.tile([P, HB, d], f16, tag="x16")
            h1 = pool16.tile([P, HB, d], f16, tag="x16")
            nc.scalar.copy(out=h0, in_=halves[0])
            nc.vector.tensor_copy(out=h1, in_=halves[1])
            return [h0, h1]

        def emit_transpose_si(h16, dst, col0, si):
            xh = h16[si // HB]
            pt = ps_small.tile([P, dc, P], f16, tag="ps_small")
            for c in range(dc):
                nc.tensor.transpose(
                    pt[:, c, :], xh[:, si % HB, c * P : (c + 1) * P], ident16
                )
            nc.vector.tensor_copy(
                out=dst[:, :, col0 + si * P : col0 + (si + 1) * P], in_=pt
            )

        def emit_transposes(h16, dst, nb):
            for si in range(tpb):
                emit_transpose_si(h16, dst, nb * KB, si)

        def emit_vcast(halves, nb):
            # fp16 cast of a value block into vnat, split ACT/DVE
            nc.scalar.copy(out=vnat[:, nb * tpb : nb * tpb + HB, :], in_=halves[0])
            nc.vector.tensor_copy(
                out=vnat[:, nb * tpb + HB : (nb + 1) * tpb, :], in_=halves[1]
            )

        # value half-loads ride the scalar queue (emitted just-in-time so they
        # never starve the key/query loads on the sync queue)
        vhalves = [None] * nqb

        def load_vblock(nb):
            halves = []
            for half in range(2):
                xh = stage.tile([P, HB, d], f32, tag="xv_nat")
                nc.scalar.dma_start(
                    out=xh,
                    in_=v_d[
                        nb * KB + half * HB * P : nb * KB + (half + 1) * HB * P, :
                    ].rearrange("(s p) d -> p s d", p=P),
                )
                halves.append(xh)
            vhalves[nb] = halves

        # ---------------- Phase 1: key side (keyt is transient) ----------------
        with (
            tc.tile_pool(name="keyt_pool", bufs=1) as keyt_pool,
            tc.tile_pool(name="stage16k", bufs=4) as stage16k,
        ):
            keyt = keyt_pool.tile([P, dc, s], f16, name="keyt_sb")

            def emit_kg(nb):
                # KG^T block: lhsT = G chunks, rhs = keyT block
                for ec in range(dc):
                    pp = ps_st.tile([P, KB], f32, tag="psum_st")
                    for c in range(dc):
                        nc.tensor.matmul(
                            pp,
                            gsb[:, c, ec * P : (ec + 1) * P],
                            keyt[:, c, nb * KB : (nb + 1) * KB],
                            start=(c == 0),
                            stop=(c == dc - 1),
                        )
                    nc.scalar.copy(out=kgt[:, ec, nb * KB : (nb + 1) * KB], in_=pp)
                # v-vector chunks: v[k] = scale * key @ (Wk^T bq)
                for si in range(tpb):
                    vp = ps_small.tile([P, 1], f32, tag="ps_small")
                    for c in range(dc):
                        nc.tensor.matmul(
                            vp,
                            keyt[:, c, nb * KB + si * P : nb * KB + (si + 1) * P],
                            w1c[:, c : c + 1],
                            start=(c == 0),
                            stop=(c == dc - 1),
                        )
                    nc.vector.tensor_copy(
                        out=vb[:, nb * tpb + si : nb * tpb + si + 1], in_=vp
                    )

            for nb in range(nqb):
                halves = load_block(k_d, nb)
                h16 = emit_cast16(halves, stage16k)
                emit_transposes(h16, keyt, nb)
                if nb == 0:
                    # G = Wk^T Wq (emitted after the first transposes so the
                    # PE can start before the weight DMAs land)
                    for dch in range(dc):
                        gp = ps_st.tile([P, d], f32, tag="psum_st")
                        for e in range(dc):
                            nc.tensor.matmul(
                                gp,
                                wkn[:, e, dch * P : (dch + 1) * P],
                                wqn[:, e, :],
                                start=(e == 0),
                                stop=(e == dc - 1),
                            )
                        nc.scalar.copy(out=gsb[:, dch, :], in_=gp)
                emit_kg(nb)

        # ---------------- Phase 2: attention (scores transposed) ----------------
        with (
            tc.tile_pool(name="expt_pool", bufs=nkc) as expt_pool,
            tc.tile_pool(name="rsum_pool", bufs=2) as rsum_pool,
            tc.tile_pool(name="unsb_pool", bufs=2) as unsb_pool,
            tc.tile_pool(name="osb_pool", bufs=2) as osb_pool,
            tc.tile_pool(name="stat_pool", bufs=2) as stat_pool,
            tc.tile_pool(name="stage16q", bufs=2) as stage16q,
            tc.tile_pool(name="ps_ut", bufs=2, space="PSUM") as ps_ut,
        ):
            # query block 0 + value block 0 before the main loop
            qhalves = load_block(q_d, 0)
            emit_transposes(emit_cast16(qhalves, stage16q), qryt, 0)
            for nb in range(min(3, nqb)):
                load_vblock(nb)
            emit_vcast(vhalves[0], 0)
            if not lazy:
                for nb in range(1, nqb):
                    emit_transposes(
                        emit_cast16(load_block(q_d, nb), stage16q), qryt, nb
                    )
                for nb in range(3, nqb):
                    load_vblock(nb)
                for nb in range(1, nqb):
                    emit_vcast(vhalves[nb], nb)

            def emit_output(qb, un_sb, rs16, recip_row):
                for qs in range(tpb):
                    rc_ps = ps_small.tile([P, 1], f32, tag="ps_small")
                    nc.tensor.transpose(
                        rc_ps,
                        recip_row[0:1, qs * P : (qs + 1) * P],
                        ident32[0:1, 0:1],
                    )
                    rc = stat_pool.tile([P, 1], f32, tag="rc")
                    nc.vector.tensor_copy(out=rc, in_=rc_ps)
                    po = ps_small.tile([P, d], f32, tag="ps_small")
                    for c in range(dc):
                        nc.tensor.matmul(
                            po,
                            un_sb[:, c, qs * P : (qs + 1) * P],
                            wvt[:, c, :],
                            start=(c == 0),
                            stop=False,
                        )
                    nc.tensor.matmul(
                        po,
                        rs16[0:1, qs * P : (qs + 1) * P],
                        bv16,
                        start=False,
                        stop=True,
                    )
                    out_sb = osb_pool.tile([P, d], f32, tag="out_sb")
                    nc.scalar.activation(
                        out=out_sb, in_=po, func=Act.Identity, scale=rc[:, 0:1]
                    )
                    nc.sync.dma_start(
                        out=out_d[qb * KB + qs * P : qb * KB + (qs + 1) * P, :],
                        in_=out_sb,
                    )

            pending = None
            next_q = None
            for qb in range(nqb):
                rsum = rsum_pool.tile([P, KB], f32, tag="rsum")
                ut_a = ps_ut.tile([P, 2, KB], f32, tag="ut")
                un_sb = unsb_pool.tile([P, dc, KB], f16, tag="un_sb")
                expts = []
                for kc in range(nkc):
                    if lazy and qb == 0 and kc >= 3 and kc % 3 == 0:
                        b = kc // 3
                        if b <= nqb - 1:
                            if b + 2 <= nqb - 1:
                                load_vblock(b + 2)
                            emit_vcast(vhalves[b], b)
                    psum_st = ps_st.tile([P, KB], f32, tag="psum_st")
                    for ec in range(dc):
                        nc.tensor.matmul(
                            psum_st,
                            kgt[:, ec, kc * P : (kc + 1) * P],
                            qryt[:, ec, qb * KB : (qb + 1) * KB],
                            start=(ec == 0),
                            stop=(ec == dc - 1),
                        )
                    expt = expt_pool.tile([P, KB], f16, tag="expt")
                    expts.append(expt)
                    nc.scalar.activation(
                        out=expt,
                        in_=psum_st,
                        func=Act.Exp,
                        scale=softmax_scale,
                        bias=vb[:, kc : kc + 1],
                    )
                    if kc == 0:
                        nc.vector.tensor_copy(out=rsum, in_=expt)
                    else:
                        nc.vector.tensor_add(rsum, rsum, expt)
                    for ec in range(2):
                        nc.tensor.matmul(
                            ut_a[:, ec, :],
                            vnat[:, kc, ec * P : (ec + 1) * P],
                            expt,
                            start=(kc == 0),
                            stop=(kc == nkc - 1),
                        )
                    if lazy:
                        if kc == 0 and qb + 1 < nqb:
                            next_q = load_block(q_d, qb + 1)
                        if kc == 1 and pending is not None:
                            emit_output(*pending)
                            pending = None
                        if kc == 3 and qb + 1 < nqb:
                            next_q16 = emit_cast16(next_q, stage16q)
                        if 4 <= kc <= 7 and qb + 1 < nqb:
                            emit_transpose_si(
                                next_q16, qryt, (qb + 1) * KB, kc - 4
                            )
                    elif kc == 1 and pending is not None:
                        emit_output(*pending)
                        pending = None
                # drain pass-A psum early (frees its slot for the next block)
                nc.vector.tensor_copy(out=un_sb[:, 0:2, :], in_=ut_a)
                # row-sums + reciprocal (overlap with pass B); the partition
                # reduce runs as a 1-cyc/row fp16 matmul on an fp16 copy
                rsum16 = rsum_pool.tile([P, KB], f16, tag="rsum16")
                nc.vector.tensor_copy(out=rsum16, in_=rsum)
                rs_ps = ps_small.tile([1, KB], f32, tag="ps_small")
                nc.tensor.matmul(rs_ps, ones_col, rsum16, start=True, stop=True)
                recip_row = stat_pool.tile([1, KB], f32, tag="recip_row")
                nc.vector.reciprocal(out=recip_row, in_=rs_ps)
                rs16 = stat_pool.tile([1, KB], f16, tag="rs16")
                nc.vector.tensor_copy(out=rs16, in_=rs_ps)
                # pass B: e-chunks 2,3 over the stored exp tiles
                ut_b = ps_ut.tile([P, 2, KB], f32, tag="ut")
                for kc in range(nkc):
                    for ec in range(2):
                        nc.tensor.matmul(
                            ut_b[:, ec, :],
                            vnat[:, kc, (2 + ec) * P : (3 + ec) * P],
                            expts[kc],
                            start=(kc == 0),
                            stop=(kc == nkc - 1),
                        )
                # drain pass-B psum, split DVE/ACT
                nc.vector.tensor_copy(out=un_sb[:, 2:3, :], in_=ut_b[:, 0:1, :])
                nc.scalar.copy(out=un_sb[:, 3:4, :], in_=ut_b[:, 1:2, :])
                pending = (qb, un_sb, rs16, recip_row)
            emit_output(*pending)

    nc.compile()
    return nc


_CACHE = {}


def _get_nc():
    if "nc" not in _CACHE:
        _CACHE["nc"] = build_attention()
    return _CACHE["nc"]


def _in_maps(query, key, value, Wq, bq, Wk, bk, Wv, bv, n_cores=NCORES):
    Wq = np.asarray(Wq, np.float32)
    Wk = np.asarray(Wk, np.float32)
    Wv = np.asarray(Wv, np.float32)
    bq = np.asarray(bq, np.float32)
    bv = np.asarray(bv, np.float32)
    wqn = Wq.astype(np.float16)
    wkn = Wk.astype(np.float16)
    wvt = np.ascontiguousarray(Wv.T).astype(np.float16)
    scale = 1.0 / math.sqrt(D)
    w1 = (scale * (Wk.T @ bq)).astype(np.float16)  # [D]
    dcn = D // P
    w1c = np.ascontiguousarray(w1.reshape(dcn, P).T)  # [P, dc]
    bv16 = bv.astype(np.float16).reshape(1, D)
    query = np.asarray(query, np.float32)
    key = np.asarray(key, np.float32)
    value = np.asarray(value, np.float32)
    return [
        {
            "query": query[i],
            "key": key[i],
            "value": value[i],
            "wqn": wqn,
            "wkn": wkn,
            "wvt": wvt,
            "w1c": w1c,
            "bv16": bv16,
        }
        for i in range(n_cores)
    ]


def _build_runner():
    """Compile once and return a callable(in_maps) -> [out per core].

    Same lowering as concourse.bass2jax.run_bass_via_pjrt, but the
    jitted shard_map executable is cached so repeat kernel() calls skip
    retracing/recompiling.
    """
    import jax
    import concourse.mybir as mybir
    from concourse import bass2jax
    from jax.experimental.shard_map import shard_map
    from jax.sharding import Mesh, PartitionSpec

    bass2jax.install_neuronx_cc_hook()
    nc = _get_nc()
    partition_name = nc.partition_id_tensor.name if nc.partition_id_tensor else None
    in_names, out_names, out_avals, zero_templates = [], [], [], []
    for alloc in nc.m.functions[0].allocations:
        if not isinstance(alloc, mybir.MemoryLocationSet):
            continue
        name = alloc.memorylocations[0].name
        if alloc.kind == "ExternalInput":
            if name != partition_name:
                in_names.append(name)
        elif alloc.kind == "ExternalOutput":
            shape = tuple(alloc.tensor_shape)
            dtype = mybir.dt.np(alloc.dtype)
            out_names.append(name)
            out_avals.append(jax.core.ShapedArray(shape, dtype))
            zero_templates.append((shape, dtype))
    n_params = len(in_names)
    n_outs = len(out_names)
    all_in_names = list(in_names) + list(out_names)
    if partition_name is not None:
        all_in_names.append(partition_name)
    donate = tuple(range(n_params, n_params + n_outs))

    def _body(*args):
        operands = list(args)
        if partition_name is not None:
            operands.append(bass2jax.partition_id_tensor())
        outs = bass2jax._bass_exec_p.bind(
            *operands,
            out_avals=tuple(out_avals),
            in_names=tuple(all_in_names),
            out_names=tuple(out_names),
            lowering_input_output_aliases=(),
            sim_require_finite=True,
            sim_require_nnan=True,
            nc=nc,
        )
        return tuple(outs)

    devices = jax.devices()[:NCORES]
    mesh = Mesh(np.asarray(devices), ("core",))
    in_specs = (PartitionSpec("core"),) * (n_params + n_outs)
    out_specs = (PartitionSpec("core"),) * n_outs
    sharded = jax.jit(
        shard_map(
            _body, mesh=mesh, in_specs=in_specs, out_specs=out_specs, check_rep=False
        ),
        donate_argnums=donate,
        keep_unused=True,
    )

    def run(in_maps):
        concat_in = [
            np.concatenate([np.asarray(m[name]) for m in in_maps], axis=0)
            for name in in_names
        ]
        concat_zeros = [
            np.zeros((NCORES * shp[0], *shp[1:]), dt) for shp, dt in zero_templates
        ]
        out_arrs = sharded(*concat_in, *concat_zeros)
        out = np.asarray(out_arrs[out_names.index("out")])
        return out.reshape(NCORES, S, D)

    return run


def _get_runner():
    if "run" not in _CACHE:
        _CACHE["run"] = _build_runner()
    return _CACHE["run"]


def kernel(query, key, value, Wq, bq, Wk, bk, Wv, bv):
    run = _get_runner()
    in_maps = _in_maps(query, key, value, Wq, bq, Wk, bk, Wv, bv)
    return run(in_maps)
